# revision 8
# baseline (speedup 1.0000x reference)
"""Trainium2 Bass kernel: step-wise linear transformer layer (fast-weight attention).

Takes FULL inputs, shards batch across 8 NeuronCores, runs a chunked
linear-attention scan per core, gathers the FULL output.

Per-core structure (2 batches x 8 heads, seq 2048, d_model 512):
  - bf16 projections on PE (q,k feature-major chunk-major; v token-major)
  - elu(x)+1 = min(exp(x),1) + relu(x)   (ACT exp/relu + DVE combine)
  - k natural + h token-major each via ONE batched xbar DMA transpose
    per block (out[p,g,f] = in[f,g,p])
  - scan in chunks of C=128, per head:
      A^T = K Q^T (masked), outT = V^T A + stateT q, stateT += K^T V
    parity-pure PSUM banks (mixing PE row-groups in one bank is fatal),
    parity-interleaved emission so the two 64-row/col groups run
    concurrently on the PE array
  - fast-weight state accumulates directly in 2 persistent PSUM banks
  - Wo projection, residual (bf16 h), LayerNorm; LN apply on GPSIMD
Host packs hT into a block-major layout so each 256-step block is a
single DMA.
"""

from contextlib import ExitStack, nullcontext

import numpy as np
import ml_dtypes

import concourse.bacc as bacc
import concourse.bass as bass
import concourse.tile as tile
from concourse import mybir
from concourse.bass_utils import run_bass_kernel_spmd
from concourse.tile import add_dep_helper

# Problem constants (hardcoded per spec)
S = 2048
B = 16
D = 512
H = 8
DH = 64
SCALE = 1.0 / (DH**0.5)
EPS = 1e-5
N_CORES = 8
NB = B // N_CORES  # 2 batches per core

F32 = mybir.dt.float32
BF16 = mybir.dt.bfloat16
AF = mybir.ActivationFunctionType
ALU = mybir.AluOpType

C = 128   # scan chunk length (timesteps)
SB = 256  # seq extent per outer block
NCH = SB // C  # chunks per block


def _chain(insts):
    """Force scheduler order among same-engine instructions (no semaphores).

    Required for grouped-PSUM accumulation: the bank's start=True matmul
    must execute before later region writes, or they land on garbage.
    Also pins the parity-interleaved emission order so paired matmuls in
    disjoint PE row/col groups actually run concurrently."""
    for a, b in zip(insts, insts[1:]):
        add_dep_helper(b.ins, a.ins, sync=False, reason="pe order")


def _ilv(a, b):
    """Interleave two lists [a0,b0,a1,b1,...]."""
    out = []
    for x, y in zip(a, b):
        out.append(x)
        out.append(y)
    return out


def build_nc(s_len=S, trivial_gamma=True, time_reps=1):
    """Build + compile the per-core Bass program (SPMD, same on all cores)."""
    n_blocks = s_len // SB
    assert s_len % SB == 0

    nc = bacc.Bacc("TRN2", target_bir_lowering=False, debug=False,
                   num_devices=N_CORES)

    # Block-packed layouts (see make_in_maps)
    hT_d = nc.dram_tensor("hTp", [n_blocks, 128, 4 * NB * SB], BF16,
                          kind="ExternalInput")
    wqt_d = nc.dram_tensor("wqt", [D, D], BF16, kind="ExternalInput")
    wkt_d = nc.dram_tensor("wkt", [D, D], BF16, kind="ExternalInput")
    wvt_d = nc.dram_tensor("wvt", [D, D], BF16, kind="ExternalInput")
    wot_d = nc.dram_tensor("wot", [D, D], BF16, kind="ExternalInput")
    mask_d = nc.dram_tensor("mask", [128, 512], F32, kind="ExternalInput")
    gamma_d = nc.dram_tensor("gamma", [D], F32, kind="ExternalInput")
    beta_d = nc.dram_tensor("beta", [D], F32, kind="ExternalInput")
    y_d = nc.dram_tensor("yp", [n_blocks, 128, NCH * NB * D], F32,
                         kind="ExternalOutput")

    with tile.TileContext(nc) as tc, ExitStack() as ctx:
        wpool = ctx.enter_context(tc.tile_pool(name="wpool", bufs=1))
        hTp = ctx.enter_context(tc.tile_pool(name="hTp", bufs=3))
        htok = ctx.enter_context(tc.tile_pool(name="htok", bufs=2))
        qkp = ctx.enter_context(tc.tile_pool(name="qkp", bufs=3))
        knp = ctx.enter_context(tc.tile_pool(name="knp", bufs=2))
        vp = ctx.enter_context(tc.tile_pool(name="vp", bufs=5))
        elup = ctx.enter_context(tc.tile_pool(name="elup", bufs=6))
        scanS = ctx.enter_context(tc.tile_pool(name="scanS", bufs=6))
        outp = ctx.enter_context(tc.tile_pool(name="outp", bufs=3))
        statep = ctx.enter_context(tc.tile_pool(name="statep", bufs=1))
        xp = ctx.enter_context(tc.tile_pool(name="xp", bufs=4))
        yblk = ctx.enter_context(tc.tile_pool(name="yblk", bufs=3))

        psP = ctx.enter_context(tc.tile_pool(name="psP", bufs=2, space="PSUM"))
        psA = ctx.enter_context(tc.tile_pool(name="psA", bufs=2, space="PSUM"))
        psO = ctx.enter_context(tc.tile_pool(name="psO", bufs=2, space="PSUM"))
        psS = ctx.enter_context(tc.tile_pool(name="psS", bufs=2, space="PSUM"))

        # ---- constants / weights (resident) ----
        def load_w(dram, tag):
            w = wpool.tile([128, 4, D], BF16, tag=tag)
            nc.sync.dma_start(out=w, in_=dram.ap().rearrange(
                "(c p) od -> p c od", p=128))
            return w

        # Pre-place the activation table covering exp+ln+copy+identity+relu
        # (set 6) — without this the auto-placement pass first-fits Exp to
        # set 0 and Ln to set 5 and thrashes ~1.3us per switch.
        nc.scalar.add_instruction(mybir.InstLoadActFuncSet(
            name=nc.get_next_instruction_name(), ins=[], outs=[],
            act_func_set_id=6))

        wq_sb = load_w(wqt_d, "wq")
        wk_sb = load_w(wkt_d, "wk")
        wv_sb = load_w(wvt_d, "wv")
        wo_sb = load_w(wot_d, "wo")
        mask4_sb = wpool.tile([128, 512], F32, tag="mask4")
        nc.sync.dma_start(out=mask4_sb, in_=mask_d.ap())
        eps_sb = wpool.tile([128, 1], F32, tag="eps")
        nc.vector.memset(eps_sb, EPS)
        if not trivial_gamma:
            gam_sb = wpool.tile([128, D], F32, tag="gam")
            g_ap = gamma_d.ap()
            nc.sync.dma_start(out=gam_sb, in_=bass.AP(
                tensor=g_ap.tensor, offset=g_ap.offset,
                ap=[[0, 128]] + list(g_ap.ap)))
            bet_sb = wpool.tile([128, D], F32, tag="bet")
            b_ap = beta_d.ap()
            nc.sync.dma_start(out=bet_sb, in_=bass.AP(
                tensor=b_ap.tensor, offset=b_ap.offset,
                ap=[[0, 128]] + list(b_ap.ap)))

        # persistent fast-weight state: stateT[j, i] for head h of batch b
        # lives at partitions (h%2)*64 + j, free slot [h//2 + 4*b, i].
        # f32 master accumulates IN PSUM (one bank per parity; each bank is
        # parity-pure: only rows par*64..par*64+63 are ever written by PE).
        stT_f32 = statep.tile([128, 4 * NB, DH], F32, tag="stf")
        stT_bf = statep.tile([128, 4 * NB, DH], BF16, tag="stb")

        loop_cm = (tc.For_i(0, time_reps, 1) if time_reps > 1
                   else nullcontext(0))
        with loop_cm:
            nc.vector.memset(stT_f32, 0.0)
            nc.vector.memset(stT_bf, 0.0)
            for j in range(n_blocks):
                # ---- single-DMA block load + on-chip transposes ----
                hT_blk = hTp.tile([128, 4, NB, SB], BF16, tag="hT")
                nc.sync.dma_start(out=hT_blk, in_=hT_d.ap()[j])
                # h token-major (for residual) via xbar transposes
                h_tok = htok.tile([128, 4, NB, NCH, C], BF16, tag="htok")
                for dc in range(4):
                    for b in range(NB):
                        for ch in range(NCH):
                            nc.sync.dma_start_transpose(
                                out=h_tok[:, dc, b, ch, :],
                                in_=hT_blk[:, dc, b, ch * C:(ch + 1) * C])
                y_sb = yblk.tile([128, NCH, NB, D], F32, tag="yb")

                # ---- projections q, k (feature-major, chunk-major) + elu ----
                qT_sb = qkp.tile([128, NB, NCH, 4, C], BF16, tag="qT")
                kT_sb = qkp.tile([128, NB, NCH, 4, C], BF16, tag="kT")
                for (w_sb, dst) in ((wq_sb, qT_sb), (wk_sb, kT_sb)):
                    for oc in range(4):
                        ps = psP.tile([128, NB * SB], F32, tag="psP")
                        for dc in range(4):
                            nc.tensor.matmul(
                                out=ps,
                                lhsT=w_sb[:, dc, oc * 128:(oc + 1) * 128],
                                rhs=hT_blk[:, dc, :, :].rearrange(
                                    "p b s -> p (b s)"),
                                start=(dc == 0), stop=(dc == 3))
                        # elu(x)+1 == min(exp(x),1) + relu(x)
                        e_bf = elup.tile([128, NB * SB], BF16, tag="ebf")
                        nc.scalar.activation(out=e_bf, in_=ps, func=AF.Exp)
                        r_bf = elup.tile([128, NB * SB], BF16, tag="rbf")
                        nc.scalar.activation(out=r_bf, in_=ps, func=AF.Relu)
                        nc.vector.scalar_tensor_tensor(
                            out=dst[:, :, :, oc, :],
                            in0=e_bf.rearrange("p (b c t) -> p b c t",
                                               b=NB, c=NCH),
                            scalar=1.0,
                            in1=r_bf.rearrange("p (b c t) -> p b c t",
                                               b=NB, c=NCH),
                            op0=ALU.min, op1=ALU.add)

                # ---- K natural via xbar transposes ----
                kn_blk = knp.tile([128, NB, NCH, 4, 128], BF16, tag="knat")
                for b in range(NB):
                    for ch in range(NCH):
                        for oc in range(4):
                            nc.sync.dma_start_transpose(
                                out=kn_blk[:, b, ch, oc, :],
                                in_=kT_sb[:, b, ch, oc, :])

                # ---- projection v (token-major) ----
                v_tiles = {}
                for b in range(NB):
                    for ch in range(NCH):
                        ps = psP.tile([128, D], F32, tag="psP")
                        for dc in range(4):
                            nc.tensor.matmul(
                                out=ps,
                                lhsT=hT_blk[:, dc, b, ch * C:(ch + 1) * C],
                                rhs=wv_sb[:, dc, :],
                                start=(dc == 0), stop=(dc == 3))
                        t = vp.tile([128, D], BF16, tag="vnat")
                        if b == 0:
                            nc.scalar.copy(out=t, in_=ps)
                        else:
                            nc.vector.tensor_copy(out=t, in_=ps)
                        v_tiles[(b, ch)] = t

                # ---- scan + output-side, per (b, chunk) ----
                outT_sb = outp.tile([128, 4, NB * SB], BF16, tag="outT")
                for b in range(NB):
                    for ch in range(NCH):
                        cols = b * SB + ch * C
                        glob_ch = j * NCH + ch
                        vt = v_tiles[(b, ch)]

                        def qslice(h):
                            return qT_sb[(h % 2) * 64:(h % 2) * 64 + 64,
                                         b, ch, h // 2, :]

                        def kslice(h):
                            return kT_sb[(h % 2) * 64:(h % 2) * 64 + 64,
                                         b, ch, h // 2, :]

                        def knslice(h):
                            return kn_blk[:, b, ch, h // 2,
                                          (h % 2) * 64:(h % 2) * 64 + 64]

                        # m1: A^T = K Q^T, grouped by head PARITY
                        # (parity-pure banks), parity-interleaved emission
                        a_ps = [psA.tile([128, 4 * C], F32, tag="psA", name="a_ps")
                                for _ in range(2)]
                        mms = {0: [], 1: []}
                        for par in range(2):
                            for hh in range(4):
                                h = 2 * hh + par
                                mms[par].append(nc.tensor.matmul(
                                    out=a_ps[par][:, hh * C:(hh + 1) * C],
                                    lhsT=kslice(h), rhs=qslice(h),
                                    start=True, stop=(hh == 3),
                                    skip_group_check=True))
                        _chain(_ilv(mms[0], mms[1]))
                        am_g = []
                        for par in range(2):
                            am = scanS.tile([128, 4 * C], BF16, tag="am")
                            nc.vector.tensor_tensor(
                                out=am, in0=a_ps[par], in1=mask4_sb,
                                op=ALU.mult)
                            am_g.append(am)

                        # m2 (+ m3 state read), grouped by parity;
                        # head h=2*hh+par -> rows par*64.., col-slice hh*C
                        o_ps = [psO.tile([128, 4 * C], F32, tag="psO", name="o_ps")
                                for _ in range(2)]
                        n_mm = 4 * (2 if glob_ch > 0 else 1)
                        mms = {0: [], 1: []}
                        for par in range(2):
                            base = par * 64
                            mm_i = 0
                            for hh in range(4):
                                h = 2 * hh + par
                                reg = o_ps[par][base:base + 64,
                                                hh * C:(hh + 1) * C]
                                am_s = am_g[par][:, hh * C:(hh + 1) * C]
                                mms[par].append(nc.tensor.matmul(
                                    out=reg, lhsT=vt[:, h * DH:(h + 1) * DH],
                                    rhs=am_s, start=True,
                                    stop=(mm_i == n_mm - 1),
                                    skip_group_check=True))
                                mm_i += 1
                                if glob_ch > 0:
                                    mms[par].append(nc.tensor.matmul(
                                        out=reg,
                                        lhsT=stT_bf[base:base + 64,
                                                    hh + 4 * b, :],
                                        rhs=qslice(h), start=False,
                                        stop=(mm_i == n_mm - 1),
                                        skip_group_check=True))
                                    mm_i += 1
                        _chain(_ilv(mms[0], mms[1]))
                        for par in range(2):
                            base = par * 64
                            src = o_ps[par][base:base + 64, :].rearrange(
                                "p (c t) -> p c t", t=C)
                            dstr = outT_sb[base:base + 64, :,
                                           cols:cols + C]
                            if par == 0:
                                nc.scalar.copy(out=dstr, in_=src)
                            else:
                                nc.vector.tensor_copy(out=dstr, in_=src)

                        # m4: stateT += K^T V (per-parity banks),
                        # parity-interleaved; f32 master in SBUF
                        d_ps = [psS.tile([128, 4 * DH], F32, tag="psS",
                                         name="d_ps") for _ in range(2)]
                        mms = {0: [], 1: []}
                        for par in range(2):
                            base = par * 64
                            for hh in range(4):
                                h = 2 * hh + par
                                mms[par].append(nc.tensor.matmul(
                                    out=d_ps[par][base:base + 64,
                                                  hh * DH:(hh + 1) * DH],
                                    lhsT=knslice(h),
                                    rhs=vt[:, h * DH:(h + 1) * DH],
                                    start=True, stop=(hh == 3),
                                    skip_group_check=True))
                        _chain(_ilv(mms[0], mms[1]))
                        for par in range(2):
                            base = par * 64
                            nc.vector.tensor_add(
                                out=stT_f32[base:base + 64,
                                            4 * b:4 * b + 4, :],
                                in0=stT_f32[base:base + 64,
                                            4 * b:4 * b + 4, :],
                                in1=d_ps[par][base:base + 64, :].rearrange(
                                    "p (c i) -> p c i", i=DH))
                        nc.scalar.copy(
                            out=stT_bf[:, 4 * b:4 * b + 4, :],
                            in_=stT_f32[:, 4 * b:4 * b + 4, :])

                        # ---- Wo projection for this tok-tile ----
                        at_ps = psP.tile([128, D], F32, tag="psP")
                        for oc in range(4):
                            nc.tensor.matmul(
                                out=at_ps,
                                lhsT=outT_sb[:, oc, cols:cols + C],
                                rhs=wo_sb[:, oc, :],
                                start=(oc == 0), stop=(oc == 3))
                        # ---- residual (bf16 h) + layernorm ----
                        x_sb = xp.tile([128, D], F32, tag="x")
                        nc.vector.tensor_add(
                            out=x_sb.rearrange("p (a t) -> p a t", a=4),
                            in0=h_tok[:, :, b, ch, :],
                            in1=at_ps.rearrange("p (a t) -> p a t", a=4))
                        stats = xp.tile([128, 6], F32, tag="stats")
                        nc.vector.bn_stats(out=stats, in_=x_sb)
                        mv = xp.tile([128, 2], F32, tag="mv")
                        nc.vector.bn_aggr(out=mv, in_=stats)
                        # rstd = exp(-0.5*ln(var+eps)) — same ACT table set
                        lnv = xp.tile([128, 1], F32, tag="lnv")
                        nc.scalar.activation(out=lnv, in_=mv[:, 1:2],
                                             func=AF.Ln, bias=eps_sb)
                        rstd = xp.tile([128, 1], F32, tag="rstd")
                        nc.scalar.activation(out=rstd, in_=lnv,
                                             func=AF.Exp, scale=-0.5)
                        nmu = xp.tile([128, 1], F32, tag="nmu")
                        nc.vector.scalar_tensor_tensor(
                            out=nmu, in0=mv[:, 0:1], scalar=-1.0, in1=rstd,
                            op0=ALU.mult, op1=ALU.mult)
                        y_slice = y_sb[:, ch, b, :]
                        # LN apply on GPSIMD (SBUF->SBUF): y = x*rstd + nmu
                        nc.gpsimd.tensor_scalar(
                            out=y_slice, in0=x_sb, scalar1=rstd,
                            scalar2=nmu, op0=ALU.mult, op1=ALU.add)
                        if not trivial_gamma:
                            nc.vector.tensor_mul(out=y_slice, in0=y_slice,
                                                 in1=gam_sb)
                            nc.vector.tensor_add(out=y_slice, in0=y_slice,
                                                 in1=bet_sb)
                nc.sync.dma_start(out=y_d.ap()[j], in_=y_sb)

    nc.compile()
    return nc


_NC_CACHE = {}


def _get_nc(s_len, trivial_gamma, time_reps=1):
    key = (s_len, trivial_gamma, time_reps)
    if key not in _NC_CACHE:
        _NC_CACHE[key] = build_nc(s_len, trivial_gamma, time_reps)
    return _NC_CACHE[key]


def make_in_maps(h, Wq, Wkv, Wo, ln_gamma, ln_beta):
    """Host-side sharding + layout prep. Returns (in_maps, trivial_gamma)."""
    s_len = h.shape[0]
    nbl = s_len // SB
    h = np.ascontiguousarray(h, dtype=np.float32)
    hT = np.ascontiguousarray(h.transpose(2, 1, 0)).astype(ml_dtypes.bfloat16)
    Wk = Wkv[:D, :]
    Wv = Wkv[D:, :]
    wqt = np.ascontiguousarray(Wq.T).astype(ml_dtypes.bfloat16)
    wkt = np.ascontiguousarray(Wk.T).astype(ml_dtypes.bfloat16)
    wvt = np.ascontiguousarray(Wv.T).astype(ml_dtypes.bfloat16)
    wot = np.ascontiguousarray(Wo.T * SCALE).astype(ml_dtypes.bfloat16)
    mask = np.tile(np.triu(np.ones((128, 128), dtype=np.float32)), (1, 4))
    gamma = np.ascontiguousarray(ln_gamma, dtype=np.float32)
    beta = np.ascontiguousarray(ln_beta, dtype=np.float32)
    trivial = bool(np.all(gamma == 1.0) and np.all(beta == 0.0))

    in_maps = []
    for c in range(N_CORES):
        bsl = slice(c * NB, (c + 1) * NB)
        # hT packed: [blocks, 128 p, (dc, b, s)]   (d = dc*128 + p)
        hTc = hT[:, bsl, :]                       # [512, NB, s]
        hTp = hTc.reshape(4, 128, NB, nbl, SB).transpose(3, 1, 0, 2, 4)
        hTp = np.ascontiguousarray(hTp.reshape(nbl, 128, 4 * NB * SB))
        in_maps.append({
            "hTp": hTp,
            "wqt": wqt, "wkt": wkt, "wvt": wvt, "wot": wot,
            "mask": mask, "gamma": gamma, "beta": beta,
        })
    return in_maps, trivial


def unpack_y(yp, s_len):
    """[blocks, 128, (ch, b, d)] -> [s, NB, D]"""
    nbl = s_len // SB
    y = yp.reshape(nbl, C, NCH, NB, D).transpose(0, 2, 1, 3, 4)
    return np.ascontiguousarray(y.reshape(s_len, NB, D))


def kernel(h, Wq, Wkv, Wo, ln_gamma, ln_beta):
    s_len = h.shape[0]
    in_maps, trivial = make_in_maps(h, Wq, Wkv, Wo, ln_gamma, ln_beta)
    nc = _get_nc(s_len, trivial)
    res = run_bass_kernel_spmd(nc, in_maps, list(range(N_CORES)))
    out = np.concatenate(
        [unpack_y(res.results[c]["yp"], s_len) for c in range(N_CORES)],
        axis=1)
    return out.astype(np.float32)


# revision 9
# speedup vs baseline: 2.2050x; 2.2050x over previous
"""Trainium2 Bass kernel: step-wise linear transformer layer (fast-weight attention).

Takes FULL inputs, shards batch across 8 NeuronCores, runs a chunked
linear-attention scan per core, gathers the FULL output.

Per-core structure (2 batches x 8 heads, seq 2048, d_model 512):
  - bf16 projections on PE (q,k feature-major chunk-major; v token-major)
  - elu(x)+1 = min(exp(x),1) + relu(x)   (ACT exp/relu + DVE combine)
  - k natural + h token-major each via ONE batched xbar DMA transpose
    per block (out[p,g,f] = in[f,g,p])
  - scan in chunks of C=128, per head:
      A^T = K Q^T (masked), outT = V^T A + stateT q, stateT += K^T V
    parity-pure PSUM banks (mixing PE row-groups in one bank is fatal),
    parity-interleaved emission so the two 64-row/col groups run
    concurrently on the PE array
  - fast-weight state accumulates directly in 2 persistent PSUM banks
  - Wo projection, residual (bf16 h), LayerNorm; LN apply on GPSIMD
Host packs hT into a block-major layout so each 256-step block is a
single DMA.
"""

from contextlib import ExitStack, nullcontext

import numpy as np
import ml_dtypes

import concourse.bacc as bacc
import concourse.bass as bass
import concourse.tile as tile
from concourse import mybir
from concourse.bass_utils import run_bass_kernel_spmd
from concourse.tile import add_dep_helper

# Problem constants (hardcoded per spec)
S = 2048
B = 16
D = 512
H = 8
DH = 64
SCALE = 1.0 / (DH**0.5)
EPS = 1e-5
N_CORES = 8
NB = B // N_CORES  # 2 batches per core

F32 = mybir.dt.float32
BF16 = mybir.dt.bfloat16
AF = mybir.ActivationFunctionType
ALU = mybir.AluOpType

C = 128   # scan chunk length (timesteps)
SB = 256  # seq extent per outer block
NCH = SB // C  # chunks per block


def _chain(insts):
    """Force scheduler order among same-engine instructions (no semaphores).

    Required for grouped-PSUM accumulation: the bank's start=True matmul
    must execute before later region writes, or they land on garbage.
    Also pins the parity-interleaved emission order so paired matmuls in
    disjoint PE row/col groups actually run concurrently."""
    for a, b in zip(insts, insts[1:]):
        add_dep_helper(b.ins, a.ins, sync=False, reason="pe order")


def _ilv(a, b):
    """Interleave two lists [a0,b0,a1,b1,...]."""
    out = []
    for x, y in zip(a, b):
        out.append(x)
        out.append(y)
    return out


def build_nc(s_len=S, trivial_gamma=True, time_reps=1):
    """Build + compile the per-core Bass program (SPMD, same on all cores)."""
    n_blocks = s_len // SB
    assert s_len % SB == 0

    nc = bacc.Bacc("TRN2", target_bir_lowering=False, debug=False,
                   num_devices=N_CORES)

    # Block-packed layouts (see make_in_maps)
    hT_d = nc.dram_tensor("hTp", [n_blocks, 128, 4 * NB * SB], BF16,
                          kind="ExternalInput")
    h_d = nc.dram_tensor("hp", [n_blocks, 128, NCH * NB * D], F32,
                         kind="ExternalInput")
    wqt_d = nc.dram_tensor("wqt", [D, D], BF16, kind="ExternalInput")
    wkt_d = nc.dram_tensor("wkt", [D, D], BF16, kind="ExternalInput")
    wvt_d = nc.dram_tensor("wvt", [D, D], BF16, kind="ExternalInput")
    wot_d = nc.dram_tensor("wot", [D, D], BF16, kind="ExternalInput")
    mask_d = nc.dram_tensor("mask", [128, 512], F32, kind="ExternalInput")
    gamma_d = nc.dram_tensor("gamma", [D], F32, kind="ExternalInput")
    beta_d = nc.dram_tensor("beta", [D], F32, kind="ExternalInput")
    y_d = nc.dram_tensor("yp", [n_blocks, 128, NCH * NB * D], F32,
                         kind="ExternalOutput")

    with tile.TileContext(nc) as tc, ExitStack() as ctx:
        wpool = ctx.enter_context(tc.tile_pool(name="wpool", bufs=1))
        hTp = ctx.enter_context(tc.tile_pool(name="hTp", bufs=3))
        htok = ctx.enter_context(tc.tile_pool(name="htok", bufs=2))
        qkp = ctx.enter_context(tc.tile_pool(name="qkp", bufs=3))
        knp = ctx.enter_context(tc.tile_pool(name="knp", bufs=2))
        vp = ctx.enter_context(tc.tile_pool(name="vp", bufs=5))
        elup = ctx.enter_context(tc.tile_pool(name="elup", bufs=6))
        scanS = ctx.enter_context(tc.tile_pool(name="scanS", bufs=6))
        outp = ctx.enter_context(tc.tile_pool(name="outp", bufs=3))
        statep = ctx.enter_context(tc.tile_pool(name="statep", bufs=1))
        xp = ctx.enter_context(tc.tile_pool(name="xp", bufs=4))
        yblk = ctx.enter_context(tc.tile_pool(name="yblk", bufs=3))

        psP = ctx.enter_context(tc.tile_pool(name="psP", bufs=2, space="PSUM"))
        psA = ctx.enter_context(tc.tile_pool(name="psA", bufs=2, space="PSUM"))
        psO = ctx.enter_context(tc.tile_pool(name="psO", bufs=2, space="PSUM"))
        psS = ctx.enter_context(tc.tile_pool(name="psS", bufs=2, space="PSUM"))

        # ---- constants / weights (resident) ----
        def load_w(dram, tag):
            w = wpool.tile([128, 4, D], BF16, tag=tag)
            nc.sync.dma_start(out=w, in_=dram.ap().rearrange(
                "(c p) od -> p c od", p=128))
            return w

        # Pre-place the activation table covering exp+ln+copy+identity+relu
        # (set 6) — without this the auto-placement pass first-fits Exp to
        # set 0 and Ln to set 5 and thrashes ~1.3us per switch.
        nc.scalar.add_instruction(mybir.InstLoadActFuncSet(
            name=nc.get_next_instruction_name(), ins=[], outs=[],
            act_func_set_id=6))

        wq_sb = load_w(wqt_d, "wq")
        wk_sb = load_w(wkt_d, "wk")
        wv_sb = load_w(wvt_d, "wv")
        wo_sb = load_w(wot_d, "wo")
        mask4_sb = wpool.tile([128, 512], F32, tag="mask4")
        nc.sync.dma_start(out=mask4_sb, in_=mask_d.ap())
        eps_sb = wpool.tile([128, 1], F32, tag="eps")
        nc.vector.memset(eps_sb, EPS)
        if not trivial_gamma:
            gam_sb = wpool.tile([128, D], F32, tag="gam")
            g_ap = gamma_d.ap()
            nc.sync.dma_start(out=gam_sb, in_=bass.AP(
                tensor=g_ap.tensor, offset=g_ap.offset,
                ap=[[0, 128]] + list(g_ap.ap)))
            bet_sb = wpool.tile([128, D], F32, tag="bet")
            b_ap = beta_d.ap()
            nc.sync.dma_start(out=bet_sb, in_=bass.AP(
                tensor=b_ap.tensor, offset=b_ap.offset,
                ap=[[0, 128]] + list(b_ap.ap)))

        # persistent fast-weight state: stateT[j, i] for head h of batch b
        # lives at partitions (h%2)*64 + j, free slot [h//2 + 4*b, i].
        # f32 master accumulates IN PSUM (one bank per parity; each bank is
        # parity-pure: only rows par*64..par*64+63 are ever written by PE).
        stT_f32 = statep.tile([128, 4 * NB, DH], F32, tag="stf")
        stT_bf = statep.tile([128, 4 * NB, DH], BF16, tag="stb")

        loop_cm = (tc.For_i(0, time_reps, 1) if time_reps > 1
                   else nullcontext(0))
        with loop_cm:
            nc.vector.memset(stT_f32, 0.0)
            nc.vector.memset(stT_bf, 0.0)
            for j in range(n_blocks):
                # ---- single-DMA block load + on-chip transposes ----
                hT_blk = hTp.tile([128, 4, NB, SB], BF16, tag="hT")
                nc.sync.dma_start(out=hT_blk, in_=hT_d.ap()[j])
                h_blk = htok.tile([128, NCH, NB, D], F32, tag="hb")
                nc.sync.dma_start(out=h_blk, in_=h_d.ap()[j])
                y_sb = yblk.tile([128, NCH, NB, D], F32, tag="yb")

                # ---- projections q, k (feature-major, chunk-major) + elu ----
                qT_sb = qkp.tile([128, NB, NCH, 4, C], BF16, tag="qT")
                kT_sb = qkp.tile([128, NB, NCH, 4, C], BF16, tag="kT")
                for (w_sb, dst) in ((wq_sb, qT_sb), (wk_sb, kT_sb)):
                    for oc in range(4):
                        ps = psP.tile([128, NB * SB], F32, tag="psP")
                        for dc in range(4):
                            nc.tensor.matmul(
                                out=ps,
                                lhsT=w_sb[:, dc, oc * 128:(oc + 1) * 128],
                                rhs=hT_blk[:, dc, :, :].rearrange(
                                    "p b s -> p (b s)"),
                                start=(dc == 0), stop=(dc == 3))
                        # elu(x)+1 == min(exp(x),1) + relu(x)
                        e_bf = elup.tile([128, NB * SB], BF16, tag="ebf")
                        nc.scalar.activation(out=e_bf, in_=ps, func=AF.Exp)
                        r_bf = elup.tile([128, NB * SB], BF16, tag="rbf")
                        nc.scalar.activation(out=r_bf, in_=ps, func=AF.Relu)
                        nc.vector.scalar_tensor_tensor(
                            out=dst[:, :, :, oc, :],
                            in0=e_bf.rearrange("p (b c t) -> p b c t",
                                               b=NB, c=NCH),
                            scalar=1.0,
                            in1=r_bf.rearrange("p (b c t) -> p b c t",
                                               b=NB, c=NCH),
                            op0=ALU.min, op1=ALU.add)

                # ---- K natural via xbar transposes ----
                kn_blk = knp.tile([128, NB, NCH, 4, 128], BF16, tag="knat")
                for b in range(NB):
                    for ch in range(NCH):
                        for oc in range(4):
                            nc.sync.dma_start_transpose(
                                out=kn_blk[:, b, ch, oc, :],
                                in_=kT_sb[:, b, ch, oc, :])

                # ---- projection v (token-major) ----
                v_tiles = {}
                for b in range(NB):
                    for ch in range(NCH):
                        ps = psP.tile([128, D], F32, tag="psP")
                        for dc in range(4):
                            nc.tensor.matmul(
                                out=ps,
                                lhsT=hT_blk[:, dc, b, ch * C:(ch + 1) * C],
                                rhs=wv_sb[:, dc, :],
                                start=(dc == 0), stop=(dc == 3))
                        t = vp.tile([128, D], BF16, tag="vnat")
                        if b == 0:
                            nc.scalar.copy(out=t, in_=ps)
                        else:
                            nc.vector.tensor_copy(out=t, in_=ps)
                        v_tiles[(b, ch)] = t

                # ---- scan + output-side, per (b, chunk) ----
                outT_sb = outp.tile([128, 4, NB * SB], BF16, tag="outT")
                for b in range(NB):
                    for ch in range(NCH):
                        cols = b * SB + ch * C
                        glob_ch = j * NCH + ch
                        vt = v_tiles[(b, ch)]

                        def qslice(h):
                            return qT_sb[(h % 2) * 64:(h % 2) * 64 + 64,
                                         b, ch, h // 2, :]

                        def kslice(h):
                            return kT_sb[(h % 2) * 64:(h % 2) * 64 + 64,
                                         b, ch, h // 2, :]

                        def knslice(h):
                            return kn_blk[:, b, ch, h // 2,
                                          (h % 2) * 64:(h % 2) * 64 + 64]

                        # m1: A^T = K Q^T, grouped by head PARITY
                        # (parity-pure banks), parity-interleaved emission
                        a_ps = [psA.tile([128, 4 * C], F32, tag="psA", name="a_ps")
                                for _ in range(2)]
                        mms = {0: [], 1: []}
                        for par in range(2):
                            for hh in range(4):
                                h = 2 * hh + par
                                mms[par].append(nc.tensor.matmul(
                                    out=a_ps[par][:, hh * C:(hh + 1) * C],
                                    lhsT=kslice(h), rhs=qslice(h),
                                    start=True, stop=(hh == 3),
                                    skip_group_check=True))
                        _chain(_ilv(mms[0], mms[1]))
                        am_g = []
                        for par in range(2):
                            am = scanS.tile([128, 4 * C], BF16, tag="am")
                            nc.vector.tensor_tensor(
                                out=am, in0=a_ps[par], in1=mask4_sb,
                                op=ALU.mult)
                            am_g.append(am)

                        # m2 (+ m3 state read), grouped by parity;
                        # head h=2*hh+par -> rows par*64.., col-slice hh*C
                        o_ps = [psO.tile([128, 4 * C], F32, tag="psO", name="o_ps")
                                for _ in range(2)]
                        n_mm = 4 * (2 if glob_ch > 0 else 1)
                        mms = {0: [], 1: []}
                        for par in range(2):
                            base = par * 64
                            mm_i = 0
                            for hh in range(4):
                                h = 2 * hh + par
                                reg = o_ps[par][base:base + 64,
                                                hh * C:(hh + 1) * C]
                                am_s = am_g[par][:, hh * C:(hh + 1) * C]
                                mms[par].append(nc.tensor.matmul(
                                    out=reg, lhsT=vt[:, h * DH:(h + 1) * DH],
                                    rhs=am_s, start=True,
                                    stop=(mm_i == n_mm - 1),
                                    skip_group_check=True))
                                mm_i += 1
                                if glob_ch > 0:
                                    mms[par].append(nc.tensor.matmul(
                                        out=reg,
                                        lhsT=stT_bf[base:base + 64,
                                                    hh + 4 * b, :],
                                        rhs=qslice(h), start=False,
                                        stop=(mm_i == n_mm - 1),
                                        skip_group_check=True))
                                    mm_i += 1
                        _chain(_ilv(mms[0], mms[1]))
                        for par in range(2):
                            base = par * 64
                            src = o_ps[par][base:base + 64, :].rearrange(
                                "p (c t) -> p c t", t=C)
                            dstr = outT_sb[base:base + 64, :,
                                           cols:cols + C]
                            if par == 0:
                                nc.scalar.copy(out=dstr, in_=src)
                            else:
                                nc.vector.tensor_copy(out=dstr, in_=src)

                        # m4: stateT += K^T V (per-parity banks),
                        # parity-interleaved; f32 master in SBUF
                        d_ps = [psS.tile([128, 4 * DH], F32, tag="psS",
                                         name="d_ps") for _ in range(2)]
                        mms = {0: [], 1: []}
                        for par in range(2):
                            base = par * 64
                            for hh in range(4):
                                h = 2 * hh + par
                                mms[par].append(nc.tensor.matmul(
                                    out=d_ps[par][base:base + 64,
                                                  hh * DH:(hh + 1) * DH],
                                    lhsT=knslice(h),
                                    rhs=vt[:, h * DH:(h + 1) * DH],
                                    start=True, stop=(hh == 3),
                                    skip_group_check=True))
                        _chain(_ilv(mms[0], mms[1]))
                        for par in range(2):
                            base = par * 64
                            nc.vector.tensor_add(
                                out=stT_f32[base:base + 64,
                                            4 * b:4 * b + 4, :],
                                in0=stT_f32[base:base + 64,
                                            4 * b:4 * b + 4, :],
                                in1=d_ps[par][base:base + 64, :].rearrange(
                                    "p (c i) -> p c i", i=DH))
                        nc.scalar.copy(
                            out=stT_bf[:, 4 * b:4 * b + 4, :],
                            in_=stT_f32[:, 4 * b:4 * b + 4, :])

                        # ---- Wo projection for this tok-tile ----
                        at_ps = psP.tile([128, D], F32, tag="psP")
                        for oc in range(4):
                            nc.tensor.matmul(
                                out=at_ps,
                                lhsT=outT_sb[:, oc, cols:cols + C],
                                rhs=wo_sb[:, oc, :],
                                start=(oc == 0), stop=(oc == 3))
                        # ---- residual (bf16 h) + layernorm ----
                        x_sb = xp.tile([128, D], F32, tag="x")
                        nc.vector.tensor_add(out=x_sb,
                                             in0=h_blk[:, ch, b, :],
                                             in1=at_ps)
                        stats = xp.tile([128, 6], F32, tag="stats")
                        nc.vector.bn_stats(out=stats, in_=x_sb)
                        mv = xp.tile([128, 2], F32, tag="mv")
                        nc.vector.bn_aggr(out=mv, in_=stats)
                        # rstd = exp(-0.5*ln(var+eps)) — same ACT table set
                        lnv = xp.tile([128, 1], F32, tag="lnv")
                        nc.scalar.activation(out=lnv, in_=mv[:, 1:2],
                                             func=AF.Ln, bias=eps_sb)
                        rstd = xp.tile([128, 1], F32, tag="rstd")
                        nc.scalar.activation(out=rstd, in_=lnv,
                                             func=AF.Exp, scale=-0.5)
                        nmu = xp.tile([128, 1], F32, tag="nmu")
                        nc.vector.scalar_tensor_tensor(
                            out=nmu, in0=mv[:, 0:1], scalar=-1.0, in1=rstd,
                            op0=ALU.mult, op1=ALU.mult)
                        y_slice = y_sb[:, ch, b, :]
                        nc.scalar.activation(out=y_slice, in_=x_sb,
                                             func=AF.Identity,
                                             bias=nmu, scale=rstd)
                        if not trivial_gamma:
                            nc.vector.tensor_mul(out=y_slice, in0=y_slice,
                                                 in1=gam_sb)
                            nc.vector.tensor_add(out=y_slice, in0=y_slice,
                                                 in1=bet_sb)
                nc.sync.dma_start(out=y_d.ap()[j], in_=y_sb)

    nc.compile()
    return nc


_NC_CACHE = {}


def _get_nc(s_len, trivial_gamma, time_reps=1):
    key = (s_len, trivial_gamma, time_reps)
    if key not in _NC_CACHE:
        _NC_CACHE[key] = build_nc(s_len, trivial_gamma, time_reps)
    return _NC_CACHE[key]


def make_in_maps(h, Wq, Wkv, Wo, ln_gamma, ln_beta):
    """Host-side sharding + layout prep. Returns (in_maps, trivial_gamma)."""
    s_len = h.shape[0]
    nbl = s_len // SB
    h = np.ascontiguousarray(h, dtype=np.float32)
    hT = np.ascontiguousarray(h.transpose(2, 1, 0)).astype(ml_dtypes.bfloat16)
    Wk = Wkv[:D, :]
    Wv = Wkv[D:, :]
    wqt = np.ascontiguousarray(Wq.T).astype(ml_dtypes.bfloat16)
    wkt = np.ascontiguousarray(Wk.T).astype(ml_dtypes.bfloat16)
    wvt = np.ascontiguousarray(Wv.T).astype(ml_dtypes.bfloat16)
    wot = np.ascontiguousarray(Wo.T * SCALE).astype(ml_dtypes.bfloat16)
    mask = np.tile(np.triu(np.ones((128, 128), dtype=np.float32)), (1, 4))
    gamma = np.ascontiguousarray(ln_gamma, dtype=np.float32)
    beta = np.ascontiguousarray(ln_beta, dtype=np.float32)
    trivial = bool(np.all(gamma == 1.0) and np.all(beta == 0.0))

    in_maps = []
    for c in range(N_CORES):
        bsl = slice(c * NB, (c + 1) * NB)
        # hT packed: [blocks, 128 p, (dc, b, s)]   (d = dc*128 + p)
        hTc = hT[:, bsl, :]                       # [512, NB, s]
        hTp = hTc.reshape(4, 128, NB, nbl, SB).transpose(3, 1, 0, 2, 4)
        hTp = np.ascontiguousarray(hTp.reshape(nbl, 128, 4 * NB * SB))
        hc = h[:, bsl, :]
        hp = hc.reshape(nbl, NCH, C, NB, D).transpose(0, 2, 1, 3, 4)
        hp = np.ascontiguousarray(hp.reshape(nbl, 128, NCH * NB * D))
        in_maps.append({
            "hTp": hTp, "hp": hp,
            "wqt": wqt, "wkt": wkt, "wvt": wvt, "wot": wot,
            "mask": mask, "gamma": gamma, "beta": beta,
        })
    return in_maps, trivial


def unpack_y(yp, s_len):
    """[blocks, 128, (ch, b, d)] -> [s, NB, D]"""
    nbl = s_len // SB
    y = yp.reshape(nbl, C, NCH, NB, D).transpose(0, 2, 1, 3, 4)
    return np.ascontiguousarray(y.reshape(s_len, NB, D))


def kernel(h, Wq, Wkv, Wo, ln_gamma, ln_beta):
    s_len = h.shape[0]
    in_maps, trivial = make_in_maps(h, Wq, Wkv, Wo, ln_gamma, ln_beta)
    nc = _get_nc(s_len, trivial)
    res = run_bass_kernel_spmd(nc, in_maps, list(range(N_CORES)))
    out = np.concatenate(
        [unpack_y(res.results[c]["yp"], s_len) for c in range(N_CORES)],
        axis=1)
    return out.astype(np.float32)


# revision 10
# speedup vs baseline: 2.7170x; 1.2322x over previous
"""Trainium2 Bass kernel: step-wise linear transformer layer (fast-weight attention).

Takes FULL inputs, shards batch across 8 NeuronCores, runs a chunked
linear-attention scan per core, gathers the FULL output.

Per-core structure (2 batches x 8 heads, seq 2048, d_model 512):
  - bf16 projections on PE (q,k feature-major chunk-major; v token-major)
  - elu(x)+1 = min(exp(x),1) + relu(x)   (ACT exp/relu + DVE combine)
  - k natural + h token-major each via ONE batched xbar DMA transpose
    per block (out[p,g,f] = in[f,g,p])
  - scan in chunks of C=128, per head:
      A^T = K Q^T (masked), outT = V^T A + stateT q, stateT += K^T V
    parity-pure PSUM banks (mixing PE row-groups in one bank is fatal),
    parity-interleaved emission so the two 64-row/col groups run
    concurrently on the PE array
  - fast-weight state accumulates directly in 2 persistent PSUM banks
  - Wo projection, residual (bf16 h), LayerNorm; LN apply on GPSIMD
Host packs hT into a block-major layout so each 256-step block is a
single DMA.
"""

from contextlib import ExitStack, nullcontext

import numpy as np
import ml_dtypes

import concourse.bacc as bacc
import concourse.bass as bass
import concourse.tile as tile
from concourse import mybir
from concourse.bass_utils import run_bass_kernel_spmd
from concourse.tile import add_dep_helper

# Problem constants (hardcoded per spec)
S = 2048
B = 16
D = 512
H = 8
DH = 64
SCALE = 1.0 / (DH**0.5)
EPS = 1e-5
N_CORES = 8
NB = B // N_CORES  # 2 batches per core

F32 = mybir.dt.float32
BF16 = mybir.dt.bfloat16
AF = mybir.ActivationFunctionType
ALU = mybir.AluOpType

C = 128   # scan chunk length (timesteps)
SB = 256  # seq extent per outer block
NCH = SB // C  # chunks per block


def _chain(insts):
    """Force scheduler order among same-engine instructions (no semaphores).

    Required for grouped-PSUM accumulation: the bank's start=True matmul
    must execute before later region writes, or they land on garbage.
    Also pins the parity-interleaved emission order so paired matmuls in
    disjoint PE row/col groups actually run concurrently."""
    for a, b in zip(insts, insts[1:]):
        add_dep_helper(b.ins, a.ins, sync=False, reason="pe order")


def _ilv(a, b):
    """Interleave two lists [a0,b0,a1,b1,...]."""
    out = []
    for x, y in zip(a, b):
        out.append(x)
        out.append(y)
    return out


def build_nc(s_len=S, trivial_gamma=True, time_reps=1):
    """Build + compile the per-core Bass program (SPMD, same on all cores)."""
    n_blocks = s_len // SB
    assert s_len % SB == 0

    nc = bacc.Bacc("TRN2", target_bir_lowering=False, debug=False,
                   num_devices=N_CORES)

    # Block-packed layouts (see make_in_maps)
    hT_d = nc.dram_tensor("hTp", [n_blocks, 128, 4 * NB * SB], BF16,
                          kind="ExternalInput")
    h_d = nc.dram_tensor("hp", [n_blocks, 128, NCH * NB * D], F32,
                         kind="ExternalInput")
    wqt_d = nc.dram_tensor("wqt", [D, D], BF16, kind="ExternalInput")
    wkt_d = nc.dram_tensor("wkt", [D, D], BF16, kind="ExternalInput")
    wvt_d = nc.dram_tensor("wvt", [D, D], BF16, kind="ExternalInput")
    wot_d = nc.dram_tensor("wot", [D, D], BF16, kind="ExternalInput")
    mask_d = nc.dram_tensor("mask", [128, 512], F32, kind="ExternalInput")
    gamma_d = nc.dram_tensor("gamma", [D], F32, kind="ExternalInput")
    beta_d = nc.dram_tensor("beta", [D], F32, kind="ExternalInput")
    y_d = nc.dram_tensor("yp", [n_blocks, 128, NCH * NB * D], F32,
                         kind="ExternalOutput")

    with tile.TileContext(nc) as tc, ExitStack() as ctx:
        wpool = ctx.enter_context(tc.tile_pool(name="wpool", bufs=1))
        hTp = ctx.enter_context(tc.tile_pool(name="hTp", bufs=3))
        htok = ctx.enter_context(tc.tile_pool(name="htok", bufs=2))
        qkp = ctx.enter_context(tc.tile_pool(name="qkp", bufs=3))
        knp = ctx.enter_context(tc.tile_pool(name="knp", bufs=2))
        vp = ctx.enter_context(tc.tile_pool(name="vp", bufs=5))
        elup = ctx.enter_context(tc.tile_pool(name="elup", bufs=6))
        scanS = ctx.enter_context(tc.tile_pool(name="scanS", bufs=6))
        outp = ctx.enter_context(tc.tile_pool(name="outp", bufs=3))
        statep = ctx.enter_context(tc.tile_pool(name="statep", bufs=1))
        xp = ctx.enter_context(tc.tile_pool(name="xp", bufs=4))
        yblk = ctx.enter_context(tc.tile_pool(name="yblk", bufs=3))

        psP = ctx.enter_context(tc.tile_pool(name="psP", bufs=2, space="PSUM"))
        psA = ctx.enter_context(tc.tile_pool(name="psA", bufs=2, space="PSUM"))
        psO = ctx.enter_context(tc.tile_pool(name="psO", bufs=2, space="PSUM"))
        psS = ctx.enter_context(tc.tile_pool(name="psS", bufs=2, space="PSUM"))

        # ---- constants / weights (resident) ----
        def load_w(dram, tag):
            w = wpool.tile([128, 4, D], BF16, tag=tag)
            nc.sync.dma_start(out=w, in_=dram.ap().rearrange(
                "(c p) od -> p c od", p=128))
            return w

        # Pre-place the activation table covering exp+ln+copy+identity+relu
        # (set 6) — without this the auto-placement pass first-fits Exp to
        # set 0 and Ln to set 5 and thrashes ~1.3us per switch.
        nc.scalar.add_instruction(mybir.InstLoadActFuncSet(
            name=nc.get_next_instruction_name(), ins=[], outs=[],
            act_func_set_id=6))

        wq_sb = load_w(wqt_d, "wq")
        wk_sb = load_w(wkt_d, "wk")
        wv_sb = load_w(wvt_d, "wv")
        wo_sb = load_w(wot_d, "wo")
        mask4_sb = wpool.tile([128, 512], F32, tag="mask4")
        nc.sync.dma_start(out=mask4_sb, in_=mask_d.ap())
        eps_sb = wpool.tile([128, 1], F32, tag="eps")
        nc.vector.memset(eps_sb, EPS)
        if not trivial_gamma:
            gam_sb = wpool.tile([128, D], F32, tag="gam")
            g_ap = gamma_d.ap()
            nc.sync.dma_start(out=gam_sb, in_=bass.AP(
                tensor=g_ap.tensor, offset=g_ap.offset,
                ap=[[0, 128]] + list(g_ap.ap)))
            bet_sb = wpool.tile([128, D], F32, tag="bet")
            b_ap = beta_d.ap()
            nc.sync.dma_start(out=bet_sb, in_=bass.AP(
                tensor=b_ap.tensor, offset=b_ap.offset,
                ap=[[0, 128]] + list(b_ap.ap)))

        # persistent fast-weight state: stateT[j, i] for head h of batch b
        # lives at partitions (h%2)*64 + j, free slot [h//2 + 4*b, i].
        # f32 master accumulates IN PSUM (one bank per parity; each bank is
        # parity-pure: only rows par*64..par*64+63 are ever written by PE).
        stT_f32 = statep.tile([128, 4 * NB, DH], F32, tag="stf")
        stT_bf = statep.tile([128, 4 * NB, DH], BF16, tag="stb")

        loop_cm = (tc.For_i(0, time_reps, 1) if time_reps > 1
                   else nullcontext(0))
        with loop_cm:
            nc.vector.memset(stT_f32, 0.0)
            nc.vector.memset(stT_bf, 0.0)
            for j in range(n_blocks):
                # ---- single-DMA block load + on-chip transposes ----
                hT_blk = hTp.tile([128, 4, NB, SB], BF16, tag="hT")
                nc.sync.dma_start(out=hT_blk, in_=hT_d.ap()[j])
                h_blk = htok.tile([128, NCH, NB, D], F32, tag="hb")
                nc.sync.dma_start(out=h_blk, in_=h_d.ap()[j])
                y_sb = yblk.tile([128, NCH, NB, D], F32, tag="yb")

                # ---- projections q, k (feature-major, chunk-major) + elu ----
                qT_sb = qkp.tile([128, NB, NCH, 4, C], BF16, tag="qT")
                kT_sb = qkp.tile([128, NB, NCH, 4, C], BF16, tag="kT")
                for (w_sb, dst) in ((wq_sb, qT_sb), (wk_sb, kT_sb)):
                    for oc in range(4):
                        ps = psP.tile([128, NB * SB], F32, tag="psP")
                        for dc in range(4):
                            nc.tensor.matmul(
                                out=ps,
                                lhsT=w_sb[:, dc, oc * 128:(oc + 1) * 128],
                                rhs=hT_blk[:, dc, :, :].rearrange(
                                    "p b s -> p (b s)"),
                                start=(dc == 0), stop=(dc == 3))
                        # elu(x)+1 == min(exp(x),1) + relu(x)
                        e_bf = elup.tile([128, NB * SB], BF16, tag="ebf")
                        nc.scalar.activation(out=e_bf, in_=ps, func=AF.Exp)
                        r_bf = elup.tile([128, NB * SB], BF16, tag="rbf")
                        nc.scalar.activation(out=r_bf, in_=ps, func=AF.Relu)
                        nc.vector.scalar_tensor_tensor(
                            out=dst[:, :, :, oc, :],
                            in0=e_bf.rearrange("p (b c t) -> p b c t",
                                               b=NB, c=NCH),
                            scalar=1.0,
                            in1=r_bf.rearrange("p (b c t) -> p b c t",
                                               b=NB, c=NCH),
                            op0=ALU.min, op1=ALU.add)

                # ---- K natural via ONE batched xbar transpose ----
                # HW-verified: out[t, (b,ch,oc), jj] = kT[jj, (b,ch,oc), t]
                kn_blk = knp.tile([128, NB, NCH, 4, 128], BF16, tag="knat")
                nc.sync.dma_start_transpose(
                    out=kn_blk.rearrange("p b c a j -> p (b c a) j"),
                    in_=kT_sb.rearrange("p b c a t -> p (b c a) t"))

                # ---- projection v (token-major) ----
                v_tiles = {}
                for b in range(NB):
                    for ch in range(NCH):
                        ps = psP.tile([128, D], F32, tag="psP")
                        for dc in range(4):
                            nc.tensor.matmul(
                                out=ps,
                                lhsT=hT_blk[:, dc, b, ch * C:(ch + 1) * C],
                                rhs=wv_sb[:, dc, :],
                                start=(dc == 0), stop=(dc == 3))
                        t = vp.tile([128, D], BF16, tag="vnat")
                        if b == 0:
                            nc.scalar.copy(out=t, in_=ps)
                        else:
                            nc.vector.tensor_copy(out=t, in_=ps)
                        v_tiles[(b, ch)] = t

                # ---- scan + output-side, per (b, chunk) ----
                outT_sb = outp.tile([128, 4, NB * SB], BF16, tag="outT")
                for b in range(NB):
                    for ch in range(NCH):
                        cols = b * SB + ch * C
                        glob_ch = j * NCH + ch
                        vt = v_tiles[(b, ch)]

                        def qslice(h):
                            return qT_sb[(h % 2) * 64:(h % 2) * 64 + 64,
                                         b, ch, h // 2, :]

                        def kslice(h):
                            return kT_sb[(h % 2) * 64:(h % 2) * 64 + 64,
                                         b, ch, h // 2, :]

                        def knslice(h):
                            return kn_blk[:, b, ch, h // 2,
                                          (h % 2) * 64:(h % 2) * 64 + 64]

                        # m1: A^T = K Q^T, grouped by head PARITY
                        # (parity-pure banks), parity-interleaved emission
                        a_ps = [psA.tile([128, 4 * C], F32, tag="psA", name="a_ps")
                                for _ in range(2)]
                        mms = {0: [], 1: []}
                        for par in range(2):
                            for hh in range(4):
                                h = 2 * hh + par
                                mms[par].append(nc.tensor.matmul(
                                    out=a_ps[par][:, hh * C:(hh + 1) * C],
                                    lhsT=kslice(h), rhs=qslice(h),
                                    start=True, stop=(hh == 3),
                                    skip_group_check=True))
                        _chain(_ilv(mms[0], mms[1]))
                        am_g = []
                        for par in range(2):
                            am = scanS.tile([128, 4 * C], BF16, tag="am")
                            nc.vector.tensor_tensor(
                                out=am, in0=a_ps[par], in1=mask4_sb,
                                op=ALU.mult)
                            am_g.append(am)

                        # m2 (+ m3 state read), grouped by parity;
                        # head h=2*hh+par -> rows par*64.., col-slice hh*C
                        o_ps = [psO.tile([128, 4 * C], F32, tag="psO", name="o_ps")
                                for _ in range(2)]
                        n_mm = 4 * (2 if glob_ch > 0 else 1)
                        mms = {0: [], 1: []}
                        for par in range(2):
                            base = par * 64
                            mm_i = 0
                            for hh in range(4):
                                h = 2 * hh + par
                                reg = o_ps[par][base:base + 64,
                                                hh * C:(hh + 1) * C]
                                am_s = am_g[par][:, hh * C:(hh + 1) * C]
                                mms[par].append(nc.tensor.matmul(
                                    out=reg, lhsT=vt[:, h * DH:(h + 1) * DH],
                                    rhs=am_s, start=True,
                                    stop=(mm_i == n_mm - 1),
                                    skip_group_check=True))
                                mm_i += 1
                                if glob_ch > 0:
                                    mms[par].append(nc.tensor.matmul(
                                        out=reg,
                                        lhsT=stT_bf[base:base + 64,
                                                    hh + 4 * b, :],
                                        rhs=qslice(h), start=False,
                                        stop=(mm_i == n_mm - 1),
                                        skip_group_check=True))
                                    mm_i += 1
                        _chain(_ilv(mms[0], mms[1]))
                        for par in range(2):
                            base = par * 64
                            src = o_ps[par][base:base + 64, :].rearrange(
                                "p (c t) -> p c t", t=C)
                            dstr = outT_sb[base:base + 64, :,
                                           cols:cols + C]
                            if par == 0:
                                nc.scalar.copy(out=dstr, in_=src)
                            else:
                                nc.vector.tensor_copy(out=dstr, in_=src)

                        # m4: stateT += K^T V (per-parity banks),
                        # parity-interleaved; f32 master in SBUF
                        d_ps = [psS.tile([128, 4 * DH], F32, tag="psS",
                                         name="d_ps") for _ in range(2)]
                        mms = {0: [], 1: []}
                        for par in range(2):
                            base = par * 64
                            for hh in range(4):
                                h = 2 * hh + par
                                mms[par].append(nc.tensor.matmul(
                                    out=d_ps[par][base:base + 64,
                                                  hh * DH:(hh + 1) * DH],
                                    lhsT=knslice(h),
                                    rhs=vt[:, h * DH:(h + 1) * DH],
                                    start=True, stop=(hh == 3),
                                    skip_group_check=True))
                        _chain(_ilv(mms[0], mms[1]))
                        for par in range(2):
                            base = par * 64
                            nc.vector.tensor_add(
                                out=stT_f32[base:base + 64,
                                            4 * b:4 * b + 4, :],
                                in0=stT_f32[base:base + 64,
                                            4 * b:4 * b + 4, :],
                                in1=d_ps[par][base:base + 64, :].rearrange(
                                    "p (c i) -> p c i", i=DH))
                        nc.scalar.copy(
                            out=stT_bf[:, 4 * b:4 * b + 4, :],
                            in_=stT_f32[:, 4 * b:4 * b + 4, :])

                        # ---- Wo projection for this tok-tile ----
                        at_ps = psP.tile([128, D], F32, tag="psP")
                        for oc in range(4):
                            nc.tensor.matmul(
                                out=at_ps,
                                lhsT=outT_sb[:, oc, cols:cols + C],
                                rhs=wo_sb[:, oc, :],
                                start=(oc == 0), stop=(oc == 3))
                        # ---- residual (bf16 h) + layernorm ----
                        x_sb = xp.tile([128, D], F32, tag="x")
                        nc.vector.tensor_add(out=x_sb,
                                             in0=h_blk[:, ch, b, :],
                                             in1=at_ps)
                        stats = xp.tile([128, 6], F32, tag="stats")
                        nc.vector.bn_stats(out=stats, in_=x_sb)
                        mv = xp.tile([128, 2], F32, tag="mv")
                        nc.vector.bn_aggr(out=mv, in_=stats)
                        # rstd = exp(-0.5*ln(var+eps)) — same ACT table set
                        lnv = xp.tile([128, 1], F32, tag="lnv")
                        nc.scalar.activation(out=lnv, in_=mv[:, 1:2],
                                             func=AF.Ln, bias=eps_sb)
                        rstd = xp.tile([128, 1], F32, tag="rstd")
                        nc.scalar.activation(out=rstd, in_=lnv,
                                             func=AF.Exp, scale=-0.5)
                        nmu = xp.tile([128, 1], F32, tag="nmu")
                        nc.vector.scalar_tensor_tensor(
                            out=nmu, in0=mv[:, 0:1], scalar=-1.0, in1=rstd,
                            op0=ALU.mult, op1=ALU.mult)
                        y_slice = y_sb[:, ch, b, :]
                        nc.scalar.activation(out=y_slice, in_=x_sb,
                                             func=AF.Identity,
                                             bias=nmu, scale=rstd)
                        if not trivial_gamma:
                            nc.vector.tensor_mul(out=y_slice, in0=y_slice,
                                                 in1=gam_sb)
                            nc.vector.tensor_add(out=y_slice, in0=y_slice,
                                                 in1=bet_sb)
                nc.sync.dma_start(out=y_d.ap()[j], in_=y_sb)

    nc.compile()
    return nc


_NC_CACHE = {}


def _get_nc(s_len, trivial_gamma, time_reps=1):
    key = (s_len, trivial_gamma, time_reps)
    if key not in _NC_CACHE:
        _NC_CACHE[key] = build_nc(s_len, trivial_gamma, time_reps)
    return _NC_CACHE[key]


def make_in_maps(h, Wq, Wkv, Wo, ln_gamma, ln_beta):
    """Host-side sharding + layout prep. Returns (in_maps, trivial_gamma)."""
    s_len = h.shape[0]
    nbl = s_len // SB
    h = np.ascontiguousarray(h, dtype=np.float32)
    hT = np.ascontiguousarray(h.transpose(2, 1, 0)).astype(ml_dtypes.bfloat16)
    Wk = Wkv[:D, :]
    Wv = Wkv[D:, :]
    wqt = np.ascontiguousarray(Wq.T).astype(ml_dtypes.bfloat16)
    wkt = np.ascontiguousarray(Wk.T).astype(ml_dtypes.bfloat16)
    wvt = np.ascontiguousarray(Wv.T).astype(ml_dtypes.bfloat16)
    wot = np.ascontiguousarray(Wo.T * SCALE).astype(ml_dtypes.bfloat16)
    mask = np.tile(np.triu(np.ones((128, 128), dtype=np.float32)), (1, 4))
    gamma = np.ascontiguousarray(ln_gamma, dtype=np.float32)
    beta = np.ascontiguousarray(ln_beta, dtype=np.float32)
    trivial = bool(np.all(gamma == 1.0) and np.all(beta == 0.0))

    in_maps = []
    for c in range(N_CORES):
        bsl = slice(c * NB, (c + 1) * NB)
        # hT packed: [blocks, 128 p, (dc, b, s)]   (d = dc*128 + p)
        hTc = hT[:, bsl, :]                       # [512, NB, s]
        hTp = hTc.reshape(4, 128, NB, nbl, SB).transpose(3, 1, 0, 2, 4)
        hTp = np.ascontiguousarray(hTp.reshape(nbl, 128, 4 * NB * SB))
        hc = h[:, bsl, :]
        hp = hc.reshape(nbl, NCH, C, NB, D).transpose(0, 2, 1, 3, 4)
        hp = np.ascontiguousarray(hp.reshape(nbl, 128, NCH * NB * D))
        in_maps.append({
            "hTp": hTp, "hp": hp,
            "wqt": wqt, "wkt": wkt, "wvt": wvt, "wot": wot,
            "mask": mask, "gamma": gamma, "beta": beta,
        })
    return in_maps, trivial


def unpack_y(yp, s_len):
    """[blocks, 128, (ch, b, d)] -> [s, NB, D]"""
    nbl = s_len // SB
    y = yp.reshape(nbl, C, NCH, NB, D).transpose(0, 2, 1, 3, 4)
    return np.ascontiguousarray(y.reshape(s_len, NB, D))


def kernel(h, Wq, Wkv, Wo, ln_gamma, ln_beta):
    s_len = h.shape[0]
    in_maps, trivial = make_in_maps(h, Wq, Wkv, Wo, ln_gamma, ln_beta)
    nc = _get_nc(s_len, trivial)
    res = run_bass_kernel_spmd(nc, in_maps, list(range(N_CORES)))
    out = np.concatenate(
        [unpack_y(res.results[c]["yp"], s_len) for c in range(N_CORES)],
        axis=1)
    return out.astype(np.float32)


# revision 11
# speedup vs baseline: 3.0964x; 1.1396x over previous
"""Trainium2 Bass kernel: step-wise linear transformer layer (fast-weight attention).

Takes FULL inputs, shards batch across 8 NeuronCores, runs a chunked
linear-attention scan per core, gathers the FULL output.

Per-core structure (2 batches x 8 heads, seq 2048, d_model 512):
  - bf16 projections on PE (q,k feature-major chunk-major; v token-major)
  - elu(x)+1 = min(exp(x),1) + relu(x)   (ACT exp/relu + DVE combine)
  - k natural + h token-major each via ONE batched xbar DMA transpose
    per block (out[p,g,f] = in[f,g,p])
  - scan in chunks of C=128, per head:
      A^T = K Q^T (masked), outT = V^T A + stateT q, stateT += K^T V
    parity-pure PSUM banks (mixing PE row-groups in one bank is fatal),
    parity-interleaved emission so the two 64-row/col groups run
    concurrently on the PE array
  - fast-weight state accumulates directly in 2 persistent PSUM banks
  - Wo projection, residual (bf16 h), LayerNorm; LN apply on GPSIMD
Host packs hT into a block-major layout so each 256-step block is a
single DMA.
"""

from contextlib import ExitStack, nullcontext

import numpy as np
import ml_dtypes

import concourse.bacc as bacc
import concourse.bass as bass
import concourse.tile as tile
from concourse import mybir
from concourse.bass_utils import run_bass_kernel_spmd
from concourse.tile import add_dep_helper

# Problem constants (hardcoded per spec)
S = 2048
B = 16
D = 512
H = 8
DH = 64
SCALE = 1.0 / (DH**0.5)
EPS = 1e-5
N_CORES = 8
NB = B // N_CORES  # 2 batches per core

F32 = mybir.dt.float32
BF16 = mybir.dt.bfloat16
AF = mybir.ActivationFunctionType
ALU = mybir.AluOpType

C = 128   # scan chunk length (timesteps)
SB = 256  # seq extent per outer block
NCH = SB // C  # chunks per block


def _chain(insts):
    """Force scheduler order among same-engine instructions (no semaphores).

    Required for grouped-PSUM accumulation: the bank's start=True matmul
    must execute before later region writes, or they land on garbage.
    Also pins the parity-interleaved emission order so paired matmuls in
    disjoint PE row/col groups actually run concurrently."""
    for a, b in zip(insts, insts[1:]):
        add_dep_helper(b.ins, a.ins, sync=False, reason="pe order")


def _ilv(a, b):
    """Interleave two lists [a0,b0,a1,b1,...]."""
    out = []
    for x, y in zip(a, b):
        out.append(x)
        out.append(y)
    return out


def build_nc(s_len=S, trivial_gamma=True, time_reps=1):
    """Build + compile the per-core Bass program (SPMD, same on all cores)."""
    n_blocks = s_len // SB
    assert s_len % SB == 0

    nc = bacc.Bacc("TRN2", target_bir_lowering=False, debug=False,
                   num_devices=N_CORES)

    # Block-packed layouts (see make_in_maps)
    hT_d = nc.dram_tensor("hTp", [n_blocks, 128, 4 * NB * SB], BF16,
                          kind="ExternalInput")
    wqt_d = nc.dram_tensor("wqt", [D, D], BF16, kind="ExternalInput")
    wkt_d = nc.dram_tensor("wkt", [D, D], BF16, kind="ExternalInput")
    wvt_d = nc.dram_tensor("wvt", [D, D], BF16, kind="ExternalInput")
    wot_d = nc.dram_tensor("wot", [D, D], BF16, kind="ExternalInput")
    mask_d = nc.dram_tensor("mask", [128, 512], F32, kind="ExternalInput")
    gamma_d = nc.dram_tensor("gamma", [D], F32, kind="ExternalInput")
    beta_d = nc.dram_tensor("beta", [D], F32, kind="ExternalInput")
    y_d = nc.dram_tensor("yp", [n_blocks, 128, NCH * NB * D], F32,
                         kind="ExternalOutput")

    with tile.TileContext(nc) as tc, ExitStack() as ctx:
        wpool = ctx.enter_context(tc.tile_pool(name="wpool", bufs=1))
        hTp = ctx.enter_context(tc.tile_pool(name="hTp", bufs=3))
        htok = ctx.enter_context(tc.tile_pool(name="htok", bufs=2))
        qkp = ctx.enter_context(tc.tile_pool(name="qkp", bufs=3))
        knp = ctx.enter_context(tc.tile_pool(name="knp", bufs=2))
        vp = ctx.enter_context(tc.tile_pool(name="vp", bufs=5))
        elup = ctx.enter_context(tc.tile_pool(name="elup", bufs=6))
        scanS = ctx.enter_context(tc.tile_pool(name="scanS", bufs=6))
        outp = ctx.enter_context(tc.tile_pool(name="outp", bufs=3))
        statep = ctx.enter_context(tc.tile_pool(name="statep", bufs=1))
        xp = ctx.enter_context(tc.tile_pool(name="xp", bufs=4))
        yblk = ctx.enter_context(tc.tile_pool(name="yblk", bufs=3))

        psP = ctx.enter_context(tc.tile_pool(name="psP", bufs=2, space="PSUM"))
        psA = ctx.enter_context(tc.tile_pool(name="psA", bufs=2, space="PSUM"))
        psO = ctx.enter_context(tc.tile_pool(name="psO", bufs=2, space="PSUM"))
        psS = ctx.enter_context(tc.tile_pool(name="psS", bufs=2, space="PSUM"))

        # ---- constants / weights (resident) ----
        def load_w(dram, tag):
            w = wpool.tile([128, 4, D], BF16, tag=tag)
            nc.sync.dma_start(out=w, in_=dram.ap().rearrange(
                "(c p) od -> p c od", p=128))
            return w

        # Pre-place the activation table covering exp+ln+copy+identity+relu
        # (set 6) — without this the auto-placement pass first-fits Exp to
        # set 0 and Ln to set 5 and thrashes ~1.3us per switch.
        nc.scalar.add_instruction(mybir.InstLoadActFuncSet(
            name=nc.get_next_instruction_name(), ins=[], outs=[],
            act_func_set_id=6))

        wq_sb = load_w(wqt_d, "wq")
        wk_sb = load_w(wkt_d, "wk")
        wv_sb = load_w(wvt_d, "wv")
        wo_sb = load_w(wot_d, "wo")
        mask4_sb = wpool.tile([128, 512], F32, tag="mask4")
        nc.sync.dma_start(out=mask4_sb, in_=mask_d.ap())
        eps_sb = wpool.tile([128, 1], F32, tag="eps")
        nc.vector.memset(eps_sb, EPS)
        if not trivial_gamma:
            gam_sb = wpool.tile([128, D], F32, tag="gam")
            g_ap = gamma_d.ap()
            nc.sync.dma_start(out=gam_sb, in_=bass.AP(
                tensor=g_ap.tensor, offset=g_ap.offset,
                ap=[[0, 128]] + list(g_ap.ap)))
            bet_sb = wpool.tile([128, D], F32, tag="bet")
            b_ap = beta_d.ap()
            nc.sync.dma_start(out=bet_sb, in_=bass.AP(
                tensor=b_ap.tensor, offset=b_ap.offset,
                ap=[[0, 128]] + list(b_ap.ap)))

        # persistent fast-weight state: stateT[j, i] for head h of batch b
        # lives at partitions (h%2)*64 + j, free slot [h//2 + 4*b, i].
        # f32 master accumulates IN PSUM (one bank per parity; each bank is
        # parity-pure: only rows par*64..par*64+63 are ever written by PE).
        stT_f32 = statep.tile([128, 4 * NB, DH], F32, tag="stf")
        stT_bf = statep.tile([128, 4 * NB, DH], BF16, tag="stb")

        loop_cm = (tc.For_i(0, time_reps, 1) if time_reps > 1
                   else nullcontext(0))
        with loop_cm:
            nc.vector.memset(stT_f32, 0.0)
            nc.vector.memset(stT_bf, 0.0)
            for j in range(n_blocks):
                # ---- single-DMA block load + on-chip transposes ----
                hT_blk = hTp.tile([128, 4, NB, SB], BF16, tag="hT")
                nc.sync.dma_start(out=hT_blk, in_=hT_d.ap()[j])
                # h token-major via batched xbar transpose (HW-verified):
                # out[t,(dc,b,ch),jj] = hT[jj,(dc,b,ch),t]
                h_tok = htok.tile([128, 4, NB, NCH, C], BF16, tag="htok")
                nc.sync.dma_start_transpose(
                    out=h_tok.rearrange("p a b c t -> p (a b c) t"),
                    in_=hT_blk.rearrange("p a b s -> p (a b s)"))
                y_sb = yblk.tile([128, NCH, NB, D], F32, tag="yb")

                # ---- projections q, k (feature-major, chunk-major) + elu ----
                qT_sb = qkp.tile([128, NB, NCH, 4, C], BF16, tag="qT")
                kT_sb = qkp.tile([128, NB, NCH, 4, C], BF16, tag="kT")
                for (w_sb, dst) in ((wq_sb, qT_sb), (wk_sb, kT_sb)):
                    for oc in range(4):
                        ps = psP.tile([128, NB * SB], F32, tag="psP")
                        for dc in range(4):
                            nc.tensor.matmul(
                                out=ps,
                                lhsT=w_sb[:, dc, oc * 128:(oc + 1) * 128],
                                rhs=hT_blk[:, dc, :, :].rearrange(
                                    "p b s -> p (b s)"),
                                start=(dc == 0), stop=(dc == 3))
                        # elu(x)+1 == min(exp(x),1) + relu(x)
                        e_bf = elup.tile([128, NB * SB], BF16, tag="ebf")
                        nc.scalar.activation(out=e_bf, in_=ps, func=AF.Exp)
                        r_bf = elup.tile([128, NB * SB], BF16, tag="rbf")
                        nc.scalar.activation(out=r_bf, in_=ps, func=AF.Relu)
                        nc.vector.scalar_tensor_tensor(
                            out=dst[:, :, :, oc, :],
                            in0=e_bf.rearrange("p (b c t) -> p b c t",
                                               b=NB, c=NCH),
                            scalar=1.0,
                            in1=r_bf.rearrange("p (b c t) -> p b c t",
                                               b=NB, c=NCH),
                            op0=ALU.min, op1=ALU.add)

                # ---- K natural via ONE batched xbar transpose ----
                # HW-verified: out[t, (b,ch,oc), jj] = kT[jj, (b,ch,oc), t]
                kn_blk = knp.tile([128, NB, NCH, 4, 128], BF16, tag="knat")
                nc.sync.dma_start_transpose(
                    out=kn_blk.rearrange("p b c a j -> p (b c a) j"),
                    in_=kT_sb.rearrange("p b c a t -> p (b c a) t"))

                # ---- projection v (token-major) ----
                v_tiles = {}
                for b in range(NB):
                    for ch in range(NCH):
                        ps = psP.tile([128, D], F32, tag="psP")
                        for dc in range(4):
                            nc.tensor.matmul(
                                out=ps,
                                lhsT=hT_blk[:, dc, b, ch * C:(ch + 1) * C],
                                rhs=wv_sb[:, dc, :],
                                start=(dc == 0), stop=(dc == 3))
                        t = vp.tile([128, D], BF16, tag="vnat")
                        if b == 0:
                            nc.scalar.copy(out=t, in_=ps)
                        else:
                            nc.vector.tensor_copy(out=t, in_=ps)
                        v_tiles[(b, ch)] = t

                # ---- scan + output-side, per (b, chunk) ----
                outT_sb = outp.tile([128, 4, NB * SB], BF16, tag="outT")
                for b in range(NB):
                    for ch in range(NCH):
                        cols = b * SB + ch * C
                        glob_ch = j * NCH + ch
                        vt = v_tiles[(b, ch)]

                        def qslice(h):
                            return qT_sb[(h % 2) * 64:(h % 2) * 64 + 64,
                                         b, ch, h // 2, :]

                        def kslice(h):
                            return kT_sb[(h % 2) * 64:(h % 2) * 64 + 64,
                                         b, ch, h // 2, :]

                        def knslice(h):
                            return kn_blk[:, b, ch, h // 2,
                                          (h % 2) * 64:(h % 2) * 64 + 64]

                        # m1: A^T = K Q^T, grouped by head PARITY
                        # (parity-pure banks), parity-interleaved emission
                        a_ps = [psA.tile([128, 4 * C], F32, tag="psA", name="a_ps")
                                for _ in range(2)]
                        mms = {0: [], 1: []}
                        for par in range(2):
                            for hh in range(4):
                                h = 2 * hh + par
                                mms[par].append(nc.tensor.matmul(
                                    out=a_ps[par][:, hh * C:(hh + 1) * C],
                                    lhsT=kslice(h), rhs=qslice(h),
                                    start=True, stop=(hh == 3),
                                    skip_group_check=True))
                        _chain(_ilv(mms[0], mms[1]))
                        am_g = []
                        for par in range(2):
                            am = scanS.tile([128, 4 * C], BF16, tag="am")
                            nc.vector.tensor_tensor(
                                out=am, in0=a_ps[par], in1=mask4_sb,
                                op=ALU.mult)
                            am_g.append(am)

                        # m2 (+ m3 state read), grouped by parity;
                        # head h=2*hh+par -> rows par*64.., col-slice hh*C
                        o_ps = [psO.tile([128, 4 * C], F32, tag="psO", name="o_ps")
                                for _ in range(2)]
                        n_mm = 4 * (2 if glob_ch > 0 else 1)
                        mms = {0: [], 1: []}
                        for par in range(2):
                            base = par * 64
                            mm_i = 0
                            for hh in range(4):
                                h = 2 * hh + par
                                reg = o_ps[par][base:base + 64,
                                                hh * C:(hh + 1) * C]
                                am_s = am_g[par][:, hh * C:(hh + 1) * C]
                                mms[par].append(nc.tensor.matmul(
                                    out=reg, lhsT=vt[:, h * DH:(h + 1) * DH],
                                    rhs=am_s, start=True,
                                    stop=(mm_i == n_mm - 1),
                                    skip_group_check=True))
                                mm_i += 1
                                if glob_ch > 0:
                                    mms[par].append(nc.tensor.matmul(
                                        out=reg,
                                        lhsT=stT_bf[base:base + 64,
                                                    hh + 4 * b, :],
                                        rhs=qslice(h), start=False,
                                        stop=(mm_i == n_mm - 1),
                                        skip_group_check=True))
                                    mm_i += 1
                        _chain(_ilv(mms[0], mms[1]))
                        for par in range(2):
                            base = par * 64
                            src = o_ps[par][base:base + 64, :].rearrange(
                                "p (c t) -> p c t", t=C)
                            dstr = outT_sb[base:base + 64, :,
                                           cols:cols + C]
                            if par == 0:
                                nc.scalar.copy(out=dstr, in_=src)
                            else:
                                nc.vector.tensor_copy(out=dstr, in_=src)

                        # m4: stateT += K^T V (per-parity banks),
                        # parity-interleaved; f32 master in SBUF
                        d_ps = [psS.tile([128, 4 * DH], F32, tag="psS",
                                         name="d_ps") for _ in range(2)]
                        mms = {0: [], 1: []}
                        for par in range(2):
                            base = par * 64
                            for hh in range(4):
                                h = 2 * hh + par
                                mms[par].append(nc.tensor.matmul(
                                    out=d_ps[par][base:base + 64,
                                                  hh * DH:(hh + 1) * DH],
                                    lhsT=knslice(h),
                                    rhs=vt[:, h * DH:(h + 1) * DH],
                                    start=True, stop=(hh == 3),
                                    skip_group_check=True))
                        _chain(_ilv(mms[0], mms[1]))
                        for par in range(2):
                            base = par * 64
                            nc.vector.tensor_add(
                                out=stT_f32[base:base + 64,
                                            4 * b:4 * b + 4, :],
                                in0=stT_f32[base:base + 64,
                                            4 * b:4 * b + 4, :],
                                in1=d_ps[par][base:base + 64, :].rearrange(
                                    "p (c i) -> p c i", i=DH))
                        nc.scalar.copy(
                            out=stT_bf[:, 4 * b:4 * b + 4, :],
                            in_=stT_f32[:, 4 * b:4 * b + 4, :])

                        # ---- Wo projection for this tok-tile ----
                        at_ps = psP.tile([128, D], F32, tag="psP")
                        for oc in range(4):
                            nc.tensor.matmul(
                                out=at_ps,
                                lhsT=outT_sb[:, oc, cols:cols + C],
                                rhs=wo_sb[:, oc, :],
                                start=(oc == 0), stop=(oc == 3))
                        # ---- residual (bf16 h) + layernorm ----
                        x_sb = xp.tile([128, D], F32, tag="x")
                        nc.vector.tensor_add(
                            out=x_sb.rearrange("p (a t) -> p a t", a=4),
                            in0=h_tok[:, :, b, ch, :],
                            in1=at_ps.rearrange("p (a t) -> p a t", a=4))
                        stats = xp.tile([128, 6], F32, tag="stats")
                        nc.vector.bn_stats(out=stats, in_=x_sb)
                        mv = xp.tile([128, 2], F32, tag="mv")
                        nc.vector.bn_aggr(out=mv, in_=stats)
                        # rstd = exp(-0.5*ln(var+eps)) — same ACT table set
                        lnv = xp.tile([128, 1], F32, tag="lnv")
                        nc.scalar.activation(out=lnv, in_=mv[:, 1:2],
                                             func=AF.Ln, bias=eps_sb)
                        rstd = xp.tile([128, 1], F32, tag="rstd")
                        nc.scalar.activation(out=rstd, in_=lnv,
                                             func=AF.Exp, scale=-0.5)
                        nmu = xp.tile([128, 1], F32, tag="nmu")
                        nc.vector.scalar_tensor_tensor(
                            out=nmu, in0=mv[:, 0:1], scalar=-1.0, in1=rstd,
                            op0=ALU.mult, op1=ALU.mult)
                        y_slice = y_sb[:, ch, b, :]
                        nc.scalar.activation(out=y_slice, in_=x_sb,
                                             func=AF.Identity,
                                             bias=nmu, scale=rstd)
                        if not trivial_gamma:
                            nc.vector.tensor_mul(out=y_slice, in0=y_slice,
                                                 in1=gam_sb)
                            nc.vector.tensor_add(out=y_slice, in0=y_slice,
                                                 in1=bet_sb)
                nc.sync.dma_start(out=y_d.ap()[j], in_=y_sb)

    nc.compile()
    return nc


_NC_CACHE = {}


def _get_nc(s_len, trivial_gamma, time_reps=1):
    key = (s_len, trivial_gamma, time_reps)
    if key not in _NC_CACHE:
        _NC_CACHE[key] = build_nc(s_len, trivial_gamma, time_reps)
    return _NC_CACHE[key]


def make_in_maps(h, Wq, Wkv, Wo, ln_gamma, ln_beta):
    """Host-side sharding + layout prep. Returns (in_maps, trivial_gamma)."""
    s_len = h.shape[0]
    nbl = s_len // SB
    h = np.ascontiguousarray(h, dtype=np.float32)
    hT = np.ascontiguousarray(h.transpose(2, 1, 0)).astype(ml_dtypes.bfloat16)
    Wk = Wkv[:D, :]
    Wv = Wkv[D:, :]
    wqt = np.ascontiguousarray(Wq.T).astype(ml_dtypes.bfloat16)
    wkt = np.ascontiguousarray(Wk.T).astype(ml_dtypes.bfloat16)
    wvt = np.ascontiguousarray(Wv.T).astype(ml_dtypes.bfloat16)
    wot = np.ascontiguousarray(Wo.T * SCALE).astype(ml_dtypes.bfloat16)
    mask = np.tile(np.triu(np.ones((128, 128), dtype=np.float32)), (1, 4))
    gamma = np.ascontiguousarray(ln_gamma, dtype=np.float32)
    beta = np.ascontiguousarray(ln_beta, dtype=np.float32)
    trivial = bool(np.all(gamma == 1.0) and np.all(beta == 0.0))

    in_maps = []
    for c in range(N_CORES):
        bsl = slice(c * NB, (c + 1) * NB)
        # hT packed: [blocks, 128 p, (dc, b, s)]   (d = dc*128 + p)
        hTc = hT[:, bsl, :]                       # [512, NB, s]
        hTp = hTc.reshape(4, 128, NB, nbl, SB).transpose(3, 1, 0, 2, 4)
        hTp = np.ascontiguousarray(hTp.reshape(nbl, 128, 4 * NB * SB))
        in_maps.append({
            "hTp": hTp,
            "wqt": wqt, "wkt": wkt, "wvt": wvt, "wot": wot,
            "mask": mask, "gamma": gamma, "beta": beta,
        })
    return in_maps, trivial


def unpack_y(yp, s_len):
    """[blocks, 128, (ch, b, d)] -> [s, NB, D]"""
    nbl = s_len // SB
    y = yp.reshape(nbl, C, NCH, NB, D).transpose(0, 2, 1, 3, 4)
    return np.ascontiguousarray(y.reshape(s_len, NB, D))


def kernel(h, Wq, Wkv, Wo, ln_gamma, ln_beta):
    s_len = h.shape[0]
    in_maps, trivial = make_in_maps(h, Wq, Wkv, Wo, ln_gamma, ln_beta)
    nc = _get_nc(s_len, trivial)
    res = run_bass_kernel_spmd(nc, in_maps, list(range(N_CORES)))
    out = np.concatenate(
        [unpack_y(res.results[c]["yp"], s_len) for c in range(N_CORES)],
        axis=1)
    return out.astype(np.float32)
